# revision 1
# baseline (speedup 1.0000x reference)
"""nn_BellmanOp (C51 categorical Bellman projection), Trainium2 Bass kernel.

out[b,k] = sum_j probs[b,j] * tri(k - clip(j + reward[b]/0.4, 0, 50))
         = interpolated 2-tap blend of the row, uniformly shifted by
           s_b = reward[b]/0.4, with out-of-range mass folded into bins 0/50.

Device algorithm (per row, n = floor(s), f = s - n):
    B[m] = (1-f)*P[m] + f*P[m-1]          (m = 0..51, P zero-padded)
    out[k] = B[k-n]                        (k = 1..49)
    out[0] = sum_{m <= -n} B[m],  out[50] = sum_{m >= 50-n} B[m]

The host sorts rows by n and pads each n-group to a 2048-row tile, so n is
uniform per tile; the per-tile shift is applied with register-offset (dynamic)
access patterns on the reads.  The kernel is fully static — one NEFF serves
any input.  Rows with |s| > NMAX (absent in practice) are fixed up exactly on
the host.  Sharding: tiles are dealt contiguously across the 8 cores
(trivially data-parallel, no communication).
"""
import sys
import numpy as np

for _p in ("/opt/trn_rl_repo", "/root/.axon_site/_ro/trn_rl_repo"):
    if _p not in sys.path:
        sys.path.append(_p)

import concourse.bass as bass
import concourse.bacc as bacc
import concourse.mybir as mybir
import concourse.tile as tile
from concourse.bass_utils import run_bass_kernel_spmd

A = 51
NMAX = 25          # max |shift| handled on device; host fixes rarer rows exactly
C = 2 * NMAX       # column where B[0] lives inside each WB block
WB = C + 2 * NMAX + A + 50
R = 16             # rows per partition per tile
TILE = 128 * R
N_CORES = 8
F32 = mybir.dt.float32
I32 = mybir.dt.int32

_NC_CACHE: dict = {}


def _build_kernel(n_tiles: int, bufs: int = 3):
    nc = bacc.Bacc("TRN2", target_bir_lowering=False, debug=False)
    probs_d = nc.dram_tensor("probs", [n_tiles, 128, R * A], F32, kind="ExternalInput")
    f_d = nc.dram_tensor("fvals", [n_tiles, 128, R], F32, kind="ExternalInput")
    meta_d = nc.dram_tensor("meta", [1, n_tiles], I32, kind="ExternalInput")
    iters_d = nc.dram_tensor("iters", [1, 1], I32, kind="ExternalInput")
    out_d = nc.dram_tensor("out", [n_tiles, 128, R * A], F32, kind="ExternalOutput")

    with tile.TileContext(nc) as tc:
        with (
            tc.tile_pool(name="pp", bufs=bufs) as pp,
            tc.tile_pool(name="xp", bufs=bufs) as xp,
            tc.tile_pool(name="fp", bufs=bufs) as fp,
            tc.tile_pool(name="mp", bufs=1) as mp,
        ):
            meta_t = mp.tile([1, n_tiles], I32)
            nc.sync.dma_start(meta_t[:], meta_d[:])
            iters_t = mp.tile([1, 1], I32)
            nc.sync.dma_start(iters_t[:], iters_d[:])

            # B blocks as raw SBUF tensors: the zero pads persist across tiles.
            b_bufs = [nc.alloc_sbuf_tensor(f"bbuf{i}", [128, R * WB], F32)
                      for i in range(2)]
            for bt in b_bufs:
                nc.vector.memset(bt.ap(), 0.0)

            _, (iters_v,) = nc.values_load_multi_w_load_instructions(
                iters_t[:1, 0:1], min_val=1, max_val=1 << 20,
                skip_runtime_bounds_check=True)

            with tc.For_i(0, iters_v, 1):
                for t in range(n_tiles):
                    pt = pp.tile([128, R * A], F32, tag="P")
                    nc.sync.dma_start(pt[:], probs_d[t])
                    ft = fp.tile([128, R], F32, tag="f")
                    nc.sync.dma_start(ft[:], f_d[t])

                    _, (rs,) = nc.values_load_multi_w_load_instructions(
                        meta_t[:1, t:t + 1], min_val=C - NMAX, max_val=C + NMAX,
                        skip_runtime_bounds_check=True,
                        engines=[mybir.EngineType.DVE, mybir.EngineType.Activation])

                    p3 = pt[:].rearrange("p (c a) -> p c a", a=A)
                    xt = xp.tile([128, R * A], F32, tag="X")
                    x3 = xt[:].rearrange("p (c a) -> p c a", a=A)
                    fb = ft[:].unsqueeze(2).broadcast_to([128, R, A])

                    # X = f * P   (GPSIMD)
                    nc.gpsimd.tensor_tensor(out=x3, in0=p3, in1=fb,
                                            op=mybir.AluOpType.mult)

                    bt = b_bufs[t % 2]
                    b3 = bt.ap().rearrange("p (c w) -> p c w", w=WB)

                    # B[C..C+50] = P - X ; B[C+51] = X[50] ; B[C+1..C+50] += X
                    nc.vector.tensor_tensor(
                        out=b3[:, :, C:C + A], in0=p3, in1=x3,
                        op=mybir.AluOpType.subtract)
                    nc.scalar.activation(
                        out=b3[:, :, C + A:C + A + 1], in_=x3[:, :, A - 1:A],
                        func=mybir.ActivationFunctionType.Copy)
                    nsplit = 30
                    nc.vector.tensor_tensor(
                        out=b3[:, :, C + 1:C + 1 + nsplit],
                        in0=b3[:, :, C + 1:C + 1 + nsplit],
                        in1=x3[:, :, 0:nsplit], op=mybir.AluOpType.add)
                    nc.gpsimd.tensor_tensor(
                        out=b3[:, :, C + 1 + nsplit:C + A],
                        in0=b3[:, :, C + 1 + nsplit:C + A],
                        in1=x3[:, :, nsplit:A - 1], op=mybir.AluOpType.add)

                    # both edge bins in one two-window reduction
                    b4 = bt.ap().rearrange("p (c w) -> p c w", w=WB).unsqueeze(2)
                    win = b4[:, :, :, bass.ds(rs - NMAX, NMAX + 1)]
                    win.ap[-2] = [NMAX + 50, 2]
                    outw = b4[:, :, :, bass.ds(rs, 1)]
                    outw.ap[-2] = [50, 2]
                    nc.vector.tensor_reduce(out=outw, in_=win,
                                            op=mybir.AluOpType.add,
                                            axis=mybir.AxisListType.X)

                    # stage the shifted window statically (dynamic-offset DMA
                    # reads go down a slow serialized path), then DMA out
                    ot = pp.tile([128, R * A], F32, tag="O")
                    o3 = ot[:].rearrange("p (c a) -> p c a", a=A)
                    nc.scalar.activation(
                        out=o3, in_=b3[:, :, bass.ds(rs, A)],
                        func=mybir.ActivationFunctionType.Copy)
                    nc.sync.dma_start(out_d[t], o3)

    nc.compile()
    return nc


def _prepare(reward: np.ndarray, probs: np.ndarray):
    bs = reward.shape[0]
    s = (reward.astype(np.float32) * np.float32(2.5)).astype(np.float32)
    s_dev = np.clip(s, -np.float32(NMAX), np.float32(NMAX)).astype(np.float32)
    n = np.floor(s_dev)
    f = (s_dev - n).astype(np.float32)
    ni = n.astype(np.int32)
    exact_rows = np.nonzero(np.abs(s) > NMAX)[0]

    order = np.argsort(ni, kind="stable")
    ni_s = ni[order]
    uniq, starts = np.unique(ni_s, return_index=True)
    starts = list(starts) + [bs]

    seg_rows, seg_n = [], []
    for gi, nv in enumerate(uniq):
        lo, hi = starts[gi], starts[gi + 1]
        cnt = hi - lo
        padded = ((cnt + TILE - 1) // TILE) * TILE
        idxs = np.full(padded, -1, dtype=np.int64)
        idxs[:cnt] = order[lo:hi]
        seg_rows.append(idxs)
        seg_n += [int(nv)] * (padded // TILE)
    slot_src = np.concatenate(seg_rows) if seg_rows else np.zeros(0, np.int64)
    n_tiles_total = len(seg_n)
    T = (n_tiles_total + N_CORES - 1) // N_CORES * N_CORES
    pad_tiles = T - n_tiles_total
    if pad_tiles:
        slot_src = np.concatenate([slot_src, np.full(pad_tiles * TILE, -1, np.int64)])
        seg_n += [0] * pad_tiles
    tiles_per_core = T // N_CORES

    probs_sorted = np.zeros((T * TILE, A), dtype=np.float32)
    f_sorted = np.zeros(T * TILE, dtype=np.float32)
    valid = slot_src >= 0
    probs_sorted[valid] = probs[slot_src[valid]]
    f_sorted[valid] = f[slot_src[valid]]
    rs_all = (C - np.asarray(seg_n, dtype=np.int32)).astype(np.int32)

    in_maps = []
    for c in range(N_CORES):
        t0, t1 = c * tiles_per_core, (c + 1) * tiles_per_core
        pc = probs_sorted[t0 * TILE:t1 * TILE].reshape(tiles_per_core, 128, R * A)
        fc = f_sorted[t0 * TILE:t1 * TILE].reshape(tiles_per_core, 128, R)
        in_maps.append({
            "probs": np.ascontiguousarray(pc),
            "fvals": np.ascontiguousarray(fc),
            "meta": np.ascontiguousarray(rs_all[t0:t1].reshape(1, tiles_per_core)),
            "iters": np.array([[1]], dtype=np.int32),
        })
    return in_maps, tiles_per_core, slot_src, valid, exact_rows


def _exact_rows(reward, probs):
    atoms = (np.float32(-10.0) + np.float32(0.4) * np.arange(A)).astype(np.float32)
    new_vals = np.clip(atoms[None, :] + reward[:, None],
                       np.float32(-10), np.float32(10)).astype(np.float32)
    idx = ((new_vals + np.float32(10)) / np.float32(0.4)).astype(np.float32)
    lower = np.floor(idx)
    upper = np.ceil(idx)
    same = lower == upper
    l_coef = np.where(same, np.float32(1), upper - idx).astype(np.float32)
    u_coef = (idx - lower).astype(np.float32)
    li = lower.astype(np.int64)
    ui = upper.astype(np.int64)
    nrow = probs.shape[0]
    rows = np.broadcast_to(np.arange(nrow)[:, None], (nrow, A))
    out = np.zeros_like(probs)
    np.add.at(out, (rows, li), l_coef * probs)
    np.add.at(out, (rows, ui), u_coef * probs)
    return out


def kernel(reward: np.ndarray, probs: np.ndarray, atom_values: np.ndarray) -> np.ndarray:
    reward = np.asarray(reward, dtype=np.float32)
    probs = np.asarray(probs, dtype=np.float32)
    bs = reward.shape[0]

    in_maps, T, slot_src, valid, exact = _prepare(reward, probs)
    nc = _NC_CACHE.get(T)
    if nc is None:
        nc = _build_kernel(T)
        _NC_CACHE[T] = nc

    res = run_bass_kernel_spmd(nc, in_maps, list(range(N_CORES)), trace=False)

    out_full = np.zeros((bs, A), dtype=np.float32)
    flat = np.concatenate(
        [res.results[c]["out"].reshape(-1, A) for c in range(N_CORES)], axis=0)
    out_full[slot_src[valid]] = flat[valid]
    if len(exact):
        out_full[exact] = _exact_rows(reward[exact], probs[exact])
    return out_full

